# revision 1
# baseline (speedup 1.0000x reference)
"""3-layer GCN forward on 8 TRN2 NeuronCores (Bass/Tile).

Per layer: out = dinv*(A @ T + T) @ W + b, with T = dinv*h, dinv=1/sqrt(deg+1),
A = binary adjacency (dupes count, no self loops), leaky_relu(0.2) between.

Device plan (per core, dst-sharded in blocks):
- per-layer gather table [NT,128] bf16 rows of T (cols 64.. zero);
  per-edge dma_gather (256B rows, int16 idx -> A/B halves by src id).
- segment sums on PE with constant per-degree block-ones matrices (nodes
  binned by in-degree per side, 128-token planes, <=8-plane batches);
  node sums dma_scatter_add'ed (unique idx per call) into per-core AGG DRAM.
- epilogue per 128-row tile: Z=(AGG_A+AGG_B+T)*dinv; PE transpose; @W; +b;
  Lrelu; *dinv -> cc_in; AllGather cc_in -> next table (Shared DRAM).
"""
import numpy as np

NEG_SLOPE = 0.2
_TIMING = False  # strip custom-DMA sems so TimelineSim can run
_SKIP_EDGE = False
_SKIP_EPI = False
_SKIP_CC = False
_SKIP_T0 = False
_SKIP_ZERO = False


class _Cfg:
    def __init__(self, n_nodes, d_in=64, d_out=4, ch_planes=64, sc_cols=32,
                 ntrash=4352):
        self.W = 8
        self.N = n_nodes
        self.D = d_in
        self.DOUT = d_out
        self.BLK = (n_nodes + self.W - 1) // self.W
        self.BP = ((self.BLK + 127) // 128) * 128
        self.NT = self.W * self.BP
        self.HALF = (self.W // 2) * self.BP
        self.SRC_SPLIT = (self.W // 2) * self.BLK
        self.TRASH_ROW = self.BLK if self.BLK < self.BP else 0
        self.NTRASH = ntrash
        self.CH_PLANES = ch_planes
        self.SC_COLS = sc_cols


def _pack_side(cfg, node_counts_max, per_core_nodes, per_core_srcs, side):
    """Shared schedule + per-core token/slot data for one side."""
    W = cfg.W
    blocks = []
    plane_cursor = 0
    col_cursor = 0
    bins = sorted(d for d, n in node_counts_max.items() if d > 0 and n > 0)
    for d in bins:
        n = node_counts_max[d]
        spp = 128 // d
        assert spp >= 1, f"degree {d} > 128 unsupported"
        nq = (spp + 31) // 32
        nlev = 3 if nq == 1 else 1   # matmul out base must be 0/32/64
        if d == 1:
            nlev = 1
        ncols_max = max(1, min(8, cfg.CH_PLANES // nlev))
        P = (n + spp - 1) // spp
        p = 0
        while p < P:
            bplanes = min(ncols_max * nlev, P - p)
            ncols = min(ncols_max, bplanes)
            mms = []
            lev, q = 0, 0
            while q < bplanes:
                nb = min(ncols, bplanes - q)
                mms.append(dict(k0=lev * 32, nb=nb,
                                plane0=plane_cursor + q))
                q += nb
                lev += 1
            blocks.append(dict(col0=col_cursor, ncols=ncols, matmuls=mms,
                               plane0=plane_cursor, planes=bplanes, d=d,
                               spp=spp))
            plane_cursor += bplanes
            col_cursor += ncols
            p += bplanes
    ntok = plane_cursor * 128
    ncols_total = col_cursor

    # plane -> (d, first node slot) in stream order
    plane_slots = [None] * plane_cursor
    cur = {d: 0 for d in bins}
    for blk in blocks:
        d, spp = blk['d'], blk['spp']
        for mm in blk['matmuls']:
            for c in range(mm['nb']):
                plane_slots[mm['plane0'] + c] = (d, cur[d])
                cur[d] += spp

    trash_g = (cfg.TRASH_ROW if side == 0 else cfg.TRASH_ROW)
    gidx, sidx = [], []
    for r in range(W):
        tok = np.full(ntok, -1, np.int64)
        for pidx in range(plane_cursor):
            d, s0 = plane_slots[pidx]
            base = pidx * 128
            nodes = per_core_nodes[r].get(d, [])
            srcs = per_core_srcs[r].get(d, [])
            spp = 128 // d
            for k in range(spp):
                slot = s0 + k
                if slot < len(nodes):
                    tok[base + k * d: base + (k + 1) * d] = srcs[slot]
        rows = np.where(
            tok >= 0,
            (tok // cfg.BLK) * cfg.BP + (tok % cfg.BLK)
            - (cfg.HALF if side == 1 else 0),
            trash_g)
        assert rows.min() >= 0 and rows.max() < 32768, (rows.min(), rows.max())
        gidx.append(rows.astype(np.int16))

        nsc = ncols_total * 128
        sl = np.full(nsc, -1, np.int64)
        for blk in blocks:
            d, spp = blk['d'], blk['spp']
            nodes = per_core_nodes[r].get(d, [])
            for mm in blk['matmuls']:
                for c in range(mm['nb']):
                    _, s0 = plane_slots[mm['plane0'] + c]
                    col = blk['col0'] + c
                    for k in range(spp):
                        if s0 + k < len(nodes):
                            sl[col * 128 + mm['k0'] + k] = nodes[s0 + k]
        ncalls = (ncols_total + cfg.SC_COLS - 1) // cfg.SC_COLS
        for call in range(ncalls):
            lo = call * cfg.SC_COLS * 128
            hi = min((call + 1) * cfg.SC_COLS * 128, nsc)
            hole = np.where(sl[lo:hi] < 0)[0]
            assert len(hole) <= cfg.NTRASH, f"trash overflow {len(hole)}"
            sl[lo + hole] = cfg.BP + np.arange(len(hole))
        sidx.append(sl.astype(np.int16))

    chunks = []
    cur_c = None
    for bi, blk in enumerate(blocks):
        if cur_c is None or cur_c['planes'] + blk['planes'] > cfg.CH_PLANES:
            cur_c = dict(plane0=blk['plane0'], planes=0, blocks=[])
            chunks.append(cur_c)
        cur_c['planes'] += blk['planes']
        cur_c['blocks'].append(bi)
    return (dict(blocks=blocks, chunks=chunks, ncols=ncols_total, ntok=ntok),
            gidx, sidx)


def _preprocess(cfg, edge_index):
    W, N, BLK = cfg.W, cfg.N, cfg.BLK
    src = np.asarray(edge_index[0], np.int64)
    dst = np.asarray(edge_index[1], np.int64)
    deg = np.bincount(dst, minlength=N).astype(np.float64) + 1.0
    dinv = (1.0 / np.sqrt(deg)).astype(np.float32)

    sides = []
    for side in (0, 1):
        mask_side = (src >= cfg.SRC_SPLIT) == (side == 1)
        ncmax = {}
        pc_nodes, pc_srcs = [], []
        for r in range(W):
            lo, hi = r * BLK, min((r + 1) * BLK, N)
            m = mask_side & (dst >= lo) & (dst < hi)
            s_r, v_r = src[m], dst[m] - lo
            order = np.argsort(v_r, kind='stable')
            s_r, v_r = s_r[order], v_r[order]
            nodes, starts, counts = np.unique(
                v_r, return_index=True, return_counts=True)
            bn, bs = {}, {}
            for j in range(len(nodes)):
                d = int(counts[j])
                bn.setdefault(d, []).append(int(nodes[j]))
                bs.setdefault(d, []).append(s_r[starts[j]:starts[j] + d])
            pc_nodes.append(bn)
            pc_srcs.append(bs)
            for d, lst in bn.items():
                ncmax[d] = max(ncmax.get(d, 0), len(lst))
        sides.append(_pack_side(cfg, ncmax, pc_nodes, pc_srcs, side))
    return dinv, sides


def _wrap16(a):
    a = np.asarray(a, np.int16)
    assert a.size % 16 == 0
    w = np.ascontiguousarray(a.reshape(-1, 16).T)
    return np.tile(w, (8, 1))


def _build(cfg, sides, s_offsets, s_total):
    import concourse.bacc as bacc
    import concourse.mybir as mybir
    import concourse.tile as tile
    import concourse.masks as masks

    D, DOUT, BP, NT = cfg.D, cfg.DOUT, cfg.BP, cfg.NT
    NTILES = BP // 128
    XT = NT // 128
    f32, bf16, i16 = mybir.dt.float32, mybir.dt.bfloat16, mybir.dt.int16
    ADD = mybir.AluOpType.add
    LRELU = mybir.ActivationFunctionType.Lrelu

    nc = bacc.Bacc(None, target_bir_lowering=False)
    x_pad = nc.dram_tensor("x_pad", [NT, D], f32, kind="ExternalInput")
    x_blk = nc.dram_tensor("x_blk", [BP, D], f32, kind="ExternalInput")
    dinv_blk = nc.dram_tensor("dinv_blk", [128, NTILES], f32, kind="ExternalInput")
    w0 = nc.dram_tensor("w0", [D, D], f32, kind="ExternalInput")
    w1 = nc.dram_tensor("w1", [D, D], f32, kind="ExternalInput")
    w2 = nc.dram_tensor("w2", [D, DOUT], f32, kind="ExternalInput")
    b01 = nc.dram_tensor("b01", [128, 2 * D], f32, kind="ExternalInput")
    b2b = nc.dram_tensor("b2b", [128, DOUT], f32, kind="ExternalInput")
    smat = nc.dram_tensor("smat", [128, s_total], bf16, kind="ExternalInput")
    gidx_in = [nc.dram_tensor(f"gidx{s}", [128, sides[s][0]['ntok'] // 16],
                              i16, kind="ExternalInput") for s in (0, 1)]
    sidx_in = [nc.dram_tensor(f"sidx{s}", [128, sides[s][0]['ncols'] * 8],
                              i16, kind="ExternalInput") for s in (0, 1)]
    outr = nc.dram_tensor("outr", [BP, DOUT], f32, kind="ExternalOutput")

    table0 = nc.dram_tensor("table0", [NT, 128], bf16)
    cc_in = nc.dram_tensor("cc_in", [BP, D], bf16)
    cc_out = nc.dram_tensor("cc_out", [NT, D], bf16, addr_space="Shared")
    table_cc = nc.dram_tensor("table_cc", [NT, 128], bf16)
    agg = [nc.dram_tensor(f"agg{s}", [BP + cfg.NTRASH, D], f32)
           for s in (0, 1)]

    with tile.TileContext(nc) as tc:
        with (
            tc.tile_pool(name="const", bufs=1) as cpool,
            tc.tile_pool(name="msg", bufs=2) as msgpool,
            tc.tile_pool(name="gix", bufs=2) as gixpool,
            tc.tile_pool(name="work", bufs=2) as work,
            tc.tile_pool(name="epi", bufs=3) as epi,
            tc.tile_pool(name="psum", bufs=3, space="PSUM") as psum,
            tc.tile_pool(name="psum_e", bufs=2, space="PSUM") as psum_e,
        ):
            gsem = [nc.alloc_semaphore(f"gsem{i}") for i in range(4)]
            gcnt = [0] * 4
            ssem = [nc.alloc_semaphore(f"ssem{i}") for i in range(3)]
            scnt = [0] * 3
            zsem = [nc.alloc_semaphore(f"zsem{i}") for i in range(3)]
            zcnt = [0] * 3
            cc_sem = nc.alloc_semaphore("cc_sem")
            cc_cnt = [0]

            ident = cpool.tile([128, 128], f32)
            masks.make_identity(nc, ident[:])
            w0t = cpool.tile([D, D], f32)
            w1t = cpool.tile([D, D], f32)
            w2t = cpool.tile([D, DOUT], f32)
            nc.sync.dma_start(w0t[:], w0[:])
            nc.sync.dma_start(w1t[:], w1[:])
            nc.sync.dma_start(w2t[:], w2[:])
            b01t = cpool.tile([128, 2 * D], f32)
            nc.sync.dma_start(b01t[:], b01[:])
            b2t = cpool.tile([128, DOUT], f32)
            nc.sync.dma_start(b2t[:], b2b[:])
            dinv_blk_t = cpool.tile([128, NTILES], f32)
            nc.sync.dma_start(dinv_blk_t[:], dinv_blk[:])
            smat_t = cpool.tile([128, s_total], bf16)
            nc.sync.dma_start(smat_t[:], smat[:])
            sixt = [cpool.tile([128, sides[s][0]['ncols'] * 8], i16,
                               tag=f"six{s}", name=f"sixt{s}")
                    for s in (0, 1)]
            nc.sync.dma_start(sixt[0][:], sidx_in[0][:])
            nc.sync.dma_start(sixt[1][:], sidx_in[1][:])
            gixt = [cpool.tile([128, sides[s][0]['ntok'] // 16], i16,
                               tag=f"gix{s}", name=f"gixt{s}")
                    for s in (0, 1)]
            nc.sync.dma_start(gixt[0][:], gidx_in[0][:])
            nc.sync.dma_start(gixt[1][:], gidx_in[1][:])
            stage = [cpool.tile([128, sides[s][0]['ncols'], D], f32,
                                tag=f"stage{s}", name=f"stage{s}")
                    for s in (0, 1)]
            nc.vector.memset(stage[0][:], 0.0)
            nc.vector.memset(stage[1][:], 0.0)

            ZW = 1024
            ztile = cpool.tile([128, ZW], f32, tag="zero")
            nc.vector.memset(ztile[:], 0.0)
            zt_bf = ztile.bitcast(bf16)

            def zero_dram(t, rows, width, dtype):
                """Zero DRAM [rows, width] (tensor or AP) via flat chunks."""
                ap2 = t if not hasattr(t, "dram_tensor_name") else t[:]
                try:
                    flat = ap2.rearrange("(p a) c -> p (a c)", p=128)
                except TypeError:
                    flat = t[:].rearrange("(p a) c -> p (a c)", p=128)
                n = flat.shape[1]
                zt = ztile if dtype == f32 else zt_bf
                zmax = ZW if dtype == f32 else 2 * ZW
                off = 0
                while off < n:
                    m = min(zmax, n - off)
                    yield nc.sync.dma_start(flat[:, off:off + m], zt[:, :m])
                    off += m

            if not _SKIP_ZERO:
                for _ in zero_dram(table0, NT, 128, bf16):
                    pass
                for _ in zero_dram(table_cc, NT, 128, bf16):
                    pass

            # T0: table0[:, :64] = bf16(x_pad) (x prescaled by dinv on host)
            G8 = 8
            assert XT % G8 == 0
            for g in range(XT // G8 if not _SKIP_T0 else 0):
                r0, r1 = g * G8 * 128, (g + 1) * G8 * 128
                xt = work.tile([128, G8, D], f32, tag="t0x")
                nc.sync.dma_start(
                    xt[:], x_pad[r0:r1, :].rearrange("(p a) c -> p a c", p=128))
                tt = work.tile([128, G8, D], bf16, tag="t0o")
                nc.vector.tensor_copy(tt[:], xt[:])
                nc.sync.dma_start(
                    table0[r0:r1, 0:D].rearrange("(p a) c -> p a c", p=128),
                    tt[:])
            G7 = max(1, NTILES // 7)
            for r0 in range(0, BP, G7 * 128):
                r1 = min(BP, r0 + G7 * 128)
                na = (r1 - r0) // 128
                xt = work.tile([128, G7, D], f32, tag="t0y")
                nc.sync.dma_start(
                    xt[:, 0:na, :],
                    x_blk[r0:r1, :].rearrange("(p a) c -> p a c", p=128))
                tt = work.tile([128, G7, D], bf16, tag="t0z")
                nc.vector.tensor_copy(tt[:, 0:na, :], xt[:, 0:na, :])
                nc.sync.dma_start(
                    cc_in[r0:r1, :].rearrange("(p a) c -> p a c", p=128),
                    tt[:, 0:na, :])

            for layer in range(3):
                table = table0 if layer == 0 else table_cc
                wt = (w0t, w1t, w2t)[layer]

                # zero AGG (Tile orders scatters after via WAW on agg)
                lsem = ssem[layer]
                for s in (0, 1):
                    for ins in zero_dram(agg[s][0:BP, :], BP, D, f32):
                        pass
                for s in (0, 1):
                    sched = sides[s][0]
                    base = table[:, :] if s == 0 else table[cfg.HALF:, :]
                    blocks = sched['blocks']
                    for ci, ch in enumerate(sched['chunks']):
                        gx = gixt[s][:, ch['plane0'] * 8:
                                     (ch['plane0'] + ch['planes']) * 8]
                        msg = msgpool.tile([128, ch['planes'], 128], bf16,
                                           tag="msg")
                        k = (layer * 5 + s * 3 + ci) % 4
                        g = nc.gpsimd.dma_gather(
                            msg[:], base, gx, ch['planes'] * 128,
                            ch['planes'] * 128, 128, single_packet=False)
                        if not _TIMING:
                            g.then_inc(gsem[k], 16)
                        gcnt[k] += 16
                        gthresh = gcnt[k]
                        for bi in ch['blocks']:
                            blk = blocks[bi]
                            if blk['d'] == 1:
                                # degree-1: token IS the sum; copy via temp
                                for mm in blk['matmuls']:
                                    p0 = mm['plane0'] - ch['plane0']
                                    tmp = work.tile([128, 8, D], f32,
                                                    tag="d1tmp")
                                    c1 = nc.vector.tensor_copy(
                                        tmp[:, 0:mm['nb'], :],
                                        msg[:, p0:p0 + mm['nb'], 0:D])
                                    if not _TIMING:
                                        c1._wait_ge(gsem[k], gthresh)
                                    c2 = nc.scalar.copy(
                                        stage[s][:, blk['col0']:
                                                 blk['col0'] + mm['nb'], :],
                                        tmp[:, 0:mm['nb'], :])
                                    if layer > 0 and not _TIMING:
                                        c2._wait_ge(ssem[layer - 1],
                                                    scnt[layer - 1])
                                continue
                            soff = s_offsets[blk['d']]
                            pt = psum.tile([128, 8, D], f32, tag="segsum")
                            nq = (blk['spp'] + 31) // 32
                            for mm in blk['matmuls']:
                                p0 = mm['plane0'] - ch['plane0']
                                for j in range(nq):
                                    sw = min(32, blk['spp'] - 32 * j)
                                    m = nc.tensor.matmul(
                                        pt[mm['k0'] + 32 * j:
                                           mm['k0'] + 32 * j + sw,
                                           0:mm['nb'], :],
                                        smat_t[:, soff + 32 * j:
                                               soff + 32 * j + sw],
                                        msg[:, p0:p0 + mm['nb'], 0:D])
                                    if not _TIMING:
                                        m._wait_ge(gsem[k], gthresh)
                            copies = [
                                (slice(mm['k0'], mm['k0'] + blk['spp']),
                                 slice(0, mm['nb']))
                                for mm in blk['matmuls']]
                            for cj, (kr, cr) in enumerate(copies):
                                dst = stage[s][kr, blk['col0'] + cr.start:
                                               blk['col0'] + cr.stop, :]
                                srcp = pt[kr, cr, :]
                                if (bi + cj) % 2 == 0:
                                    c_ins = nc.vector.tensor_copy(dst, srcp)
                                else:
                                    c_ins = nc.scalar.copy(dst, srcp)
                                if layer > 0 and not _TIMING:
                                    c_ins._wait_ge(ssem[layer - 1],
                                                   scnt[layer - 1])
                    nco = sched['ncols']
                    ncall = (nco + cfg.SC_COLS - 1) // cfg.SC_COLS
                    for call in range(ncall):
                        c0 = call * cfg.SC_COLS
                        ncc = min(cfg.SC_COLS, nco - c0)
                        sc = nc.gpsimd.dma_scatter_add(
                            agg[s][:, :], stage[s][:, c0:c0 + ncc, :],
                            sixt[s][:, c0 * 8:(c0 + ncc) * 8],
                            ncc * 128, ncc * 128, D, single_packet=False)
                        if scnt[layer] and not _TIMING:
                            sc._wait_ge(lsem, scnt[layer])
                        if not _TIMING:
                            sc.then_inc(lsem, 16)
                        scnt[layer] += 16

                sc_thresh = scnt[layer]
                # epilogue, batched in groups of EG row-tiles.
                # Row mapping within a group: row = g*EG*128 + p*EG + a
                # (partition p, slice a); dinv_blk input uses same layout.
                EG = 7 if NTILES % 7 == 0 else 1
                NGRP = NTILES // EG
                for g in range(NGRP if not _SKIP_EPI else 0):
                    r0, r1 = g * EG * 128, (g + 1) * EG * 128
                    a0 = epi.tile([128, EG, D], f32, tag="a0")
                    a1 = epi.tile([128, EG, D], f32, tag="a1")
                    d0 = nc.sync.dma_start(
                        a0[:], agg[0][r0:r1, :].rearrange(
                            "(p a) c -> p a c", p=128))
                    d1 = nc.sync.dma_start(
                        a1[:], agg[1][r0:r1, :].rearrange(
                            "(p a) c -> p a c", p=128))
                    if not _TIMING:
                        d0._wait_ge(lsem, sc_thresh)
                        d1._wait_ge(lsem, sc_thresh)
                    tb = epi.tile([128, EG, D], bf16, tag="tb")
                    nc.sync.dma_start(
                        tb[:], cc_in[r0:r1, :].rearrange(
                            "(p a) c -> p a c", p=128))
                    tf = epi.tile([128, EG, D], f32, tag="tf")
                    nc.vector.tensor_copy(tf[:], tb[:])
                    z = epi.tile([128, EG, D], f32, tag="z")
                    nc.vector.tensor_add(z[:], a0[:], a1[:])
                    nc.vector.tensor_add(z[:], z[:], tf[:])
                    if layer < 2:
                        ot = epi.tile([128, EG, D], bf16, tag="tn")
                    else:
                        ot = epi.tile([128, EG, DOUT], f32, tag="o2")
                    for a in range(EG):
                        zs = z[:, a, :]
                        nc.vector.tensor_scalar_mul(
                            zs, zs, dinv_blk_t[:, g * EG + a:g * EG + a + 1])
                        ztp = psum_e.tile([D, 128], f32, tag="ztp")
                        nc.tensor.transpose(ztp[:], zs, ident[:])
                        zts = epi.tile([D, 128], f32, tag="zts")
                        nc.vector.tensor_copy(zts[:], ztp[:])
                        if layer < 2:
                            op = psum_e.tile([128, D], f32, tag="op")
                            nc.tensor.matmul(op[:], zts[:], wt[:])
                            h = epi.tile([128, D], f32, tag="h")
                            nc.vector.tensor_add(
                                h[:], op[:],
                                b01t[:, layer * D:(layer + 1) * D])
                            hl = epi.tile([128, D], f32, tag="hl")
                            nc.scalar.mul(hl[:], h[:], NEG_SLOPE)
                            nc.vector.tensor_max(hl[:], hl[:], h[:])
                            nc.vector.tensor_scalar_mul(
                                ot[:, a, :], hl[:],
                                dinv_blk_t[:, g * EG + a:g * EG + a + 1])
                        else:
                            op = psum_e.tile([128, DOUT], f32, tag="op")
                            nc.tensor.matmul(op[:], zts[:], w2t[:])
                            nc.vector.tensor_add(ot[:, a, :], op[:], b2t[:])
                    if layer < 2:
                        nc.sync.dma_start(
                            cc_in[r0:r1, :].rearrange("(p a) c -> p a c",
                                                      p=128), ot[:])
                    else:
                        nc.sync.dma_start(
                            outr[r0:r1, :].rearrange("(p a) c -> p a c",
                                                     p=128), ot[:])

                if layer < 2 and not _SKIP_CC:
                    with tc.tile_critical():
                        for kk in range(4):
                            if gcnt[kk] and not _TIMING:
                                nc.gpsimd.wait_ge(gsem[kk], gcnt[kk])
                        cci = nc.gpsimd.collective_compute(
                            "AllGather", mybir.AluOpType.bypass,
                            ins=[cc_in[:, :]], outs=[cc_out[:, :]],
                            replica_groups=[list(range(cfg.W))])
                        cci.then_inc(cc_sem, 1)
                        cc_cnt[0] += 1
                        nc.gpsimd.wait_ge(cc_sem, cc_cnt[0])
                    nc.sync.dma_start(
                        table_cc[:, 0:D].rearrange("(p a) c -> p a c", p=128),
                        cc_out[:, :].rearrange("(p a) c -> p a c", p=128))
    nc.compile()
    return nc


_CACHE = {}


def _get_program(key, cfg, edge_index):
    if key in _CACHE:
        return _CACHE[key]
    dinv, sides = _preprocess(cfg, edge_index)
    degs = sorted({blk['d'] for sched, _, _ in sides
                   for blk in sched['blocks']})
    s_offsets, off = {}, 0
    for d in degs:
        s_offsets[d] = off
        off += 128 // d
    s_total = max(16, ((off + 15) // 16) * 16)
    smat = np.zeros((128, s_total), np.float32)
    for d in degs:
        spp = 128 // d
        for t in range(spp * d):
            smat[t, s_offsets[d] + t // d] = 1.0
    import ml_dtypes
    smat = smat.astype(ml_dtypes.bfloat16)
    nc = _build(cfg, sides, s_offsets, s_total)
    _CACHE[key] = (nc, dinv, sides, smat)
    return _CACHE[key]


def kernel(x, edge_index, W0, b0, W1, b1, W2, b2, _cfg=None, _sim=False):
    import ml_dtypes
    x = np.asarray(x, np.float32)
    edge_index = np.asarray(edge_index)
    N, D = x.shape
    DOUT = np.asarray(W2).shape[1]
    cfg = _cfg or _Cfg(N, D, DOUT)
    nc, dinv, sides, smat = _get_program(
        (N, edge_index.shape[1]), cfg, edge_index)

    BP, BLK, NT, Wc = cfg.BP, cfg.BLK, cfg.NT, cfg.W
    XT, NTILES = NT // 128, BP // 128

    xs = x * dinv[:, None]
    x_pad = np.zeros((NT, D), np.float32)
    for s in range(Wc):
        lo, hi = s * BLK, min((s + 1) * BLK, N)
        x_pad[s * BP:s * BP + (hi - lo)] = xs[lo:hi]

    b01 = np.zeros((128, 2 * D), np.float32)
    b01[:, :D] = np.asarray(b0, np.float32)[None, :]
    b01[:, D:] = np.asarray(b1, np.float32)[None, :]
    b2t = np.tile(np.asarray(b2, np.float32)[None, :], (128, 1))

    in_maps = []
    for r in range(Wc):
        lo, hi = r * BLK, min((r + 1) * BLK, N)
        xb = np.zeros((BP, D), np.float32)
        xb[:hi - lo] = xs[lo:hi]
        db = np.zeros(BP, np.float32)
        db[:hi - lo] = dinv[lo:hi]
        EG = 7 if NTILES % 7 == 0 else 1
        # [p, g*EG+a] = db[g*EG*128 + p*EG + a]
        dinv_blk = np.ascontiguousarray(
            db.reshape(NTILES // EG, 128, EG).transpose(1, 0, 2)
            .reshape(128, NTILES))
        im = dict(
            x_pad=x_pad, x_blk=xb,
            dinv_blk=dinv_blk.astype(np.float32),
            w0=np.asarray(W0, np.float32), w1=np.asarray(W1, np.float32),
            w2=np.asarray(W2, np.float32), b01=b01, b2b=b2t,
            smat=smat,
            gidx0=_wrap16(sides[0][1][r]), gidx1=_wrap16(sides[1][1][r]),
            sidx0=_wrap16(sides[0][2][r]), sidx1=_wrap16(sides[1][2][r]),
        )
        in_maps.append(im)

    if _sim:
        from concourse import bass_interp
        sim = bass_interp.MultiCoreSim(nc, Wc)
        for r in range(Wc):
            for k, v in in_maps[r].items():
                sim.cores[r].tensor(k)[:] = v
            sim.cores[r].mem_tensor("outr")[:] = 0
        sim.simulate()
        results = [np.array(sim.cores[r].mem_tensor("outr")).reshape(BP, DOUT)
                   for r in range(Wc)]
    else:
        from concourse.bass_utils import run_bass_kernel_spmd
        res = run_bass_kernel_spmd(nc, in_maps, list(range(Wc)))
        results = [res.results[r]["outr"] for r in range(Wc)]

    out = np.zeros((N, DOUT), np.float32)
    for r in range(Wc):
        lo, hi = r * BLK, min((r + 1) * BLK, N)
        out[lo:hi] = results[r][:hi - lo]
    return out



# revision 20
# speedup vs baseline: 1.5772x; 1.5772x over previous
"""3-layer GCN forward on 8 TRN2 NeuronCores (Bass/Tile), v2.

Math: per layer, out = dinv * ((A+I) @ T) @ W + b with T = dinv*h,
dinv = 1/sqrt(deg+1); leaky_relu(0.2) between layers. Self-loops are
ordinary tokens (node gathers its own T row), so there is no separate
+T path.

Device plan (dst-sharded, one SPMD program on 8 cores):
- Feature table = pair-packed DRAM [NPAIR, 128] bf16: pair row p holds
  nodes 2p and 2p+1 (64 feats each). All pair rows fit int16, so one
  dma_gather base covers the whole graph (no A/B halves). Layer 0's
  table is a host-staged input (x prescaled by dinv, bf16); later
  tables are the AllGather outputs themselves - no copies, no zeroing.
- Tokens (edges + self-loops) are grouped by dst tile (128 nodes) and
  by src parity (which 64-col half of the gathered pair row), padded
  to 128-token planes shared across cores.
- Segment sums on PE: per plane, matmul(lhsT=msg[:,pl,q*64:+64],
  rhs=onehot[128,128]) accumulates into a PSUM window [64, 128] per
  dst tile (start/stop over the tile's planes). Onehot built on DVE:
  is_equal(iota_row, sid) where sid[p, pl] = dst slot of token, -1 pads.
- Epilogue per tile: psum_h = matmul(lhsT=stage[:, tile], rhs=W);
  x dinv, +b, leaky, x dinv -> bf16 cc_in (node-major [BP,64], whose
  bytes are exactly the pair view [BP/2, 128]); AllGather -> next table.
- Host: balanced node->tile permutation (equalizes tokens per
  (tile,parity) so cross-core shared plane counts stay tight); output
  unpermuted on host.
"""
import numpy as np

NEG_SLOPE = 0.2
_TIMING = False  # strip custom-DMA sems so TimelineSim can run
_DEBUG = False


class _Cfg:
    def __init__(self, n_nodes, d_in=64, d_out=4, sup_tiles=7):
        self.W = 8
        self.N = n_nodes
        self.D = d_in
        self.DOUT = d_out
        self.BLK = (n_nodes + self.W - 1) // self.W
        self.BP = ((self.BLK + 127) // 128) * 128
        self.NT = self.W * self.BP
        self.NPAIR = self.NT // 2
        self.NTILES = self.BP // 128
        # super-tile size (tiles per gather/psum chunk)
        self.SUP = sup_tiles if self.NTILES % sup_tiles == 0 else 1
        self.NSUP = self.NTILES // self.SUP
        assert self.NPAIR <= 32768, self.NPAIR


def _balance_tiles(cfg, tok_counts):
    """Assign this core's nodes (local ids) to (tile, slot) so that
    per-(tile,parity) token counts are even. tok_counts: [nloc, 2] int.
    Returns perm[nloc] -> tile*128+slot."""
    nloc = tok_counts.shape[0]
    ntiles = cfg.NTILES
    cap = np.full(ntiles, 128, np.int64)
    # leave fake slots distributed: capacity 128 each, total >= nloc
    load = np.zeros((ntiles, 2), np.float64)
    order = np.argsort(-(tok_counts.sum(1)))
    perm = np.zeros(nloc, np.int64)
    slots_used = np.zeros(ntiles, np.int64)
    for n in order:
        t0, t1 = tok_counts[n]
        # pick open tile minimizing resulting max-parity load
        best, bestv = -1, None
        cand = np.where(slots_used < cap)[0]
        v = np.maximum(load[cand, 0] + t0, load[cand, 1] + t1)
        best = cand[np.argmin(v)]
        perm[n] = best * 128 + slots_used[best]
        slots_used[best] += 1
        load[best, 0] += t0
        load[best, 1] += t1
    return perm


def _preprocess(cfg, edge_index):
    """Build shared plane schedule + per-core gidx/sid + per-core node
    permutations and dinv."""
    W, N, BLK, BP = cfg.W, cfg.N, cfg.BLK, cfg.BP
    src = np.asarray(edge_index[0], np.int64)
    dst = np.asarray(edge_index[1], np.int64)
    deg = np.bincount(dst, minlength=N).astype(np.float64) + 1.0
    dinv = (1.0 / np.sqrt(deg)).astype(np.float32)

    # per-core local node -> global row permutation
    perms = []        # per core: local node i -> row offset within block
    core_edges = []   # per core: (src_global, dst_local)
    for c in range(W):
        lo, hi = c * BLK, min((c + 1) * BLK, N)
        m = (dst >= lo) & (dst < hi)
        s_c, d_c = src[m], dst[m] - lo
        core_edges.append((s_c, d_c))
        nloc = hi - lo
        # token counts per (node, parity-of-src-row); self token too
        # NOTE: parity depends on src global ROW, which depends on the
        # src core's permutation -> chicken & egg. Use src node id
        # parity as proxy for balancing only (exact counts computed
        # later once all perms are fixed; parity imbalance is tiny).
        tc = np.zeros((nloc, 2), np.int64)
        np.add.at(tc, (d_c, s_c % 2), 1)
        own = np.arange(nloc)
        tc[own, own % 2] += 1  # self token (proxy parity)
        perms.append(_balance_tiles(cfg, tc))

    # global row of node n
    grow = np.zeros(N, np.int64)
    for c in range(W):
        lo, hi = c * BLK, min((c + 1) * BLK, N)
        grow[lo:hi] = c * BP + perms[c]

    # build token lists per core: (pair, parity, tile, slot)
    # shared plane counts: planes[t][q] = max over cores
    per_core_tok = []
    for c in range(W):
        lo, hi = c * BLK, min((c + 1) * BLK, N)
        s_c, d_c = core_edges[c]
        own = np.arange(hi - lo)
        s_all = np.concatenate([s_c, own + lo])      # self tokens
        d_all = np.concatenate([d_c, own])
        r = grow[s_all]
        pair, par = r >> 1, r & 1
        pos = perms[c][d_all]
        tile, slot = pos >> 7, pos & 127
        per_core_tok.append((pair, par, tile, slot))

    ntiles = cfg.NTILES
    counts = np.zeros((W, ntiles, 2), np.int64)
    for c in range(W):
        _, par, tile, _ = per_core_tok[c]
        np.add.at(counts[c], (tile, par), 1)
    planes_tq = (counts.max(0) + 127) // 128  # [ntiles, 2]
    planes_tq = np.maximum(planes_tq, 1)

    # canonical plane order: for g in sup, for q in (0,1),
    #   for t in tiles(g), planes(t,q)
    plane_tile = []   # per plane: (tile, q)
    chunk_meta = []   # per chunk: (plane0, nplanes, g, q)
    for g in range(cfg.NSUP):
        tl = range(g * cfg.SUP, (g + 1) * cfg.SUP)
        for q in (0, 1):
            p0 = len(plane_tile)
            for t in tl:
                for _ in range(planes_tq[t, q]):
                    plane_tile.append((t, q))
            chunk_meta.append((p0, len(plane_tile) - p0, g, q))
    nplanes = len(plane_tile)
    ntok = nplanes * 128

    # per-core gidx/sid fill
    gidxs, sids = [], []
    for c in range(W):
        pair, par, tile, slot = per_core_tok[c]
        gi = np.zeros(ntok, np.int64)          # pad -> pair row 0
        sd = np.full(ntok, -1.0, np.float32)   # pad -> no slot
        # bucket tokens by (tile, q), fill planes in canonical order
        plane_base = {}
        off = 0
        for pl, (t, q) in enumerate(plane_tile):
            plane_base.setdefault((t, q), []).append(pl)
        key = tile * 2 + par
        order = np.argsort(key, kind='stable')
        ks, ps, ss = key[order], pair[order], slot[order]
        bounds = np.searchsorted(ks, np.arange(ntiles * 2 + 1))
        for t in range(ntiles):
            for q in (0, 1):
                a, b = bounds[t * 2 + q], bounds[t * 2 + q + 1]
                cnt = b - a
                pls = plane_base[(t, q)]
                assert cnt <= len(pls) * 128, (c, t, q, cnt)
                for j, pl in enumerate(pls):
                    u, v = a + j * 128, min(a + (j + 1) * 128, b)
                    if u >= v:
                        break
                    base = pl * 128
                    gi[base:base + (v - u)] = ps[u:v]
                    sd[base:base + (v - u)] = ss[u:v]
        gidxs.append(gi.astype(np.int16))
        sids.append(sd)
    sched = dict(nplanes=nplanes, ntok=ntok, plane_tile=plane_tile,
                 chunks=chunk_meta, planes_tq=planes_tq)
    return dinv, perms, sched, gidxs, sids


def _wrap16(a):
    a = np.asarray(a, np.int16)
    assert a.size % 16 == 0
    w = np.ascontiguousarray(a.reshape(-1, 16).T)
    return np.tile(w, (8, 1))


def _build(cfg, sched):
    import concourse.bacc as bacc
    import concourse.mybir as mybir
    import concourse.tile as tile
    import concourse.masks as masks

    D, DOUT = cfg.D, cfg.DOUT
    BP, NPAIR, NTILES = cfg.BP, cfg.NPAIR, cfg.NTILES
    SUP, NSUP = cfg.SUP, cfg.NSUP
    SUPN = SUP * 128  # nodes per super-tile
    f32, bf16, i16 = mybir.dt.float32, mybir.dt.bfloat16, mybir.dt.int16
    EQ = mybir.AluOpType.is_equal
    nplanes = sched['nplanes']
    plane_tile = sched['plane_tile']
    chunks = sched['chunks']

    nc = bacc.Bacc(None, target_bir_lowering=False)
    x_table = nc.dram_tensor("x_table", [NPAIR, 128], bf16,
                             kind="ExternalInput")
    w0 = nc.dram_tensor("w0", [D, D], f32, kind="ExternalInput")
    w1 = nc.dram_tensor("w1", [D, D], f32, kind="ExternalInput")
    w2 = nc.dram_tensor("w2", [D, DOUT], f32, kind="ExternalInput")
    b01 = nc.dram_tensor("b01", [128, 2 * D], f32, kind="ExternalInput")
    b2b = nc.dram_tensor("b2b", [128, DOUT], f32, kind="ExternalInput")
    dinv_in = nc.dram_tensor("dinv_blk", [128, NTILES], f32,
                             kind="ExternalInput")
    iota_in = nc.dram_tensor("iota_rep", [128, 128], f32,
                             kind="ExternalInput")
    gidx_in = nc.dram_tensor("gidx", [128, sched['ntok'] // 16], i16,
                             kind="ExternalInput")
    sid_in = nc.dram_tensor("sid", [128, nplanes], f32,
                            kind="ExternalInput")
    outr = nc.dram_tensor("outr", [BP, DOUT], f32, kind="ExternalOutput")

    cc_in = nc.dram_tensor("cc_in", [BP, D], bf16)
    table0 = nc.dram_tensor("table0", [NPAIR, 128], bf16)
    cc_out = [nc.dram_tensor(f"cc_out{i}", [NPAIR, 128], bf16)
              for i in (0, 1)]
    dbg = [nc.dram_tensor(f"dbg{i}", [NPAIR, 128], bf16,
                          kind="ExternalOutput") for i in (0, 1)] \
        if _DEBUG else None
    dbg_st = nc.dram_tensor("dbg_st", [128, NTILES * D], f32,
                            kind="ExternalOutput") if _DEBUG else None
    np0 = chunks[0][1]
    dbg_oh = nc.dram_tensor("dbg_oh", [128, np0 * 128], bf16,
                            kind="ExternalOutput") if _DEBUG else None
    dbg_msg = nc.dram_tensor("dbg_msg", [128, np0 * 128], bf16,
                             kind="ExternalOutput") if _DEBUG else None

    with tile.TileContext(nc) as tc:
        with (
            tc.tile_pool(name="const", bufs=1) as cpool,
            tc.tile_pool(name="msg", bufs=3) as msgpool,
            tc.tile_pool(name="oh", bufs=3) as ohpool,
            tc.tile_pool(name="stage", bufs=2) as stpool,
            tc.tile_pool(name="epi", bufs=3) as epi,
            tc.tile_pool(name="psum", bufs=2, space="PSUM") as psum,
            tc.tile_pool(name="psum_e", bufs=2, space="PSUM") as psum_e,
        ):
            gsem = [nc.alloc_semaphore(f"gsem{i}") for i in range(4)]
            gcnt = [0] * 4
            cc_sem = nc.alloc_semaphore("cc_sem")
            cc_cnt = [0]

            w0t = cpool.tile([D, D], f32)
            w1t = cpool.tile([D, D], f32)
            w2t = cpool.tile([D, DOUT], f32)
            nc.sync.dma_start(w0t[:], w0[:])
            nc.sync.dma_start(w1t[:], w1[:])
            nc.sync.dma_start(w2t[:], w2[:])
            b01t = cpool.tile([128, 2 * D], f32)
            nc.sync.dma_start(b01t[:], b01[:])
            b2t = cpool.tile([128, DOUT], f32)
            nc.sync.dma_start(b2t[:], b2b[:])
            dinvt = cpool.tile([128, NTILES], f32)
            nc.sync.dma_start(dinvt[:], dinv_in[:])
            gixt = cpool.tile([128, sched['ntok'] // 16], i16)
            nc.sync.dma_start(gixt[:], gidx_in[:])
            sidt = cpool.tile([128, nplanes], f32)
            nc.sync.dma_start(sidt[:], sid_in[:])
            iot = cpool.tile([128, 128], f32)
            nc.sync.dma_start(iot[:], iota_in[:])
            nc.sync.dma_start(table0[:], x_table[:])
            ident = cpool.tile([128, 128], f32)
            masks.make_identity(nc, ident[:])

            from concourse.bass import AP

            def iota_bcast(np_):
                """[128, np_, 128] view of iot with stride-0 middle dim."""
                a = iot[:]
                return AP(a.tensor, a.offset,
                          [list(a.ap[0]), [0, np_], list(a.ap[1])])

            def sid_bcast(p0, np_):
                """[128, np_, 128] view of sidt cols p0.. with stride-0
                last dim."""
                a = sidt[:, p0:p0 + np_]
                return AP(a.tensor, a.offset,
                          [list(a.ap[0]), list(a.ap[1]), [0, 128]])

            for layer in range(3):
                table = table0 if layer == 0 else cc_out[layer - 1]
                wt = (w0t, w1t, w2t)[layer]
                stage = stpool.tile([128, NTILES * D], f32, tag="stage")

                for g in range(NSUP):
                    # two parity chunks for this super-tile
                    cms, ohs, thr, sems = [], [], [], []
                    for q in (0, 1):
                        p0, np_, gg, qq = chunks[g * 2 + q]
                        assert (gg, qq) == (g, q)
                        msg = msgpool.tile([128, np_ * 128], bf16,
                                           tag="msg")
                        k = (layer * 2 * NSUP + g * 2 + q) % 4
                        gx = gixt[:, p0 * 8:(p0 + np_) * 8]
                        gth = nc.gpsimd.dma_gather(
                            msg[:].rearrange("p (a c) -> p a c", c=128),
                            table[:, :], gx, np_ * 128,
                            np_ * 128, 128, single_packet=False)
                        if not _TIMING:
                            gth.then_inc(gsem[k], 16)
                        gcnt[k] += 16
                        oh = ohpool.tile([128, np_ * 128], bf16, tag="oh")
                        for j in range(np_):
                            nc.vector.tensor_scalar(
                                oh[:, j * 128:(j + 1) * 128], iot[:],
                                sidt[:, p0 + j:p0 + j + 1], None, EQ)
                        if _DEBUG and layer == 0 and g == 0 and q == 0:
                            dmc = nc.sync.dma_start(dbg_msg[:], msg[:])
                            if not _TIMING:
                                dmc._wait_ge(gsem[k], gcnt[k])
                            nc.sync.dma_start(dbg_oh[:], oh[:])
                        cms.append((p0, np_, msg))
                        ohs.append(oh)
                        thr.append(gcnt[k])
                        sems.append(gsem[k])

                    pg = psum.tile([128, 512], f32, tag="pg")
                    pls = []
                    for trel in range(SUP):
                        t = g * SUP + trel
                        for q in (0, 1):
                            p0, np_, _ = cms[q]
                            for j in range(np_):
                                if plane_tile[p0 + j][0] == t:
                                    pls.append((trel, q, j))
                    for i, (trel, q, j) in enumerate(pls):
                        _, _, msg = cms[q]
                        mm = nc.tensor.matmul(
                            pg[:, trel * D:(trel + 1) * D],
                            ohs[q][:, j * 128:(j + 1) * 128],
                            msg[:, j * 128 + q * D:
                                j * 128 + q * D + D],
                            start=(i == 0), stop=(i == len(pls) - 1))
                        if not _TIMING:
                            mm._wait_ge(sems[q], thr[q])
                    nc.vector.tensor_copy(
                        stage[:, g * SUP * D:(g + 1) * SUP * D],
                        pg[:, 0:SUP * D])

                if _DEBUG and layer == 2:
                    nc.sync.dma_start(dbg_st[:, 0:NTILES * D],
                                      stage[:, 0:NTILES * D])
                # epilogue per super-tile
                for g in range(NSUP):
                    if layer < 2:
                        ob = epi.tile([128, SUP, D], bf16, tag="ob")
                    else:
                        ob = epi.tile([128, SUP, DOUT], f32, tag="ob2")
                    for trel in range(SUP):
                        t = g * SUP + trel
                        ztp = psum_e.tile([128, 512], f32, tag="ztp")
                        nc.tensor.transpose(
                            ztp[0:D, 0:128], stage[:, t * D:(t + 1) * D],
                            ident[:])
                        zts = epi.tile([D, 128], f32, tag="zts")
                        nc.vector.tensor_copy(zts[:], ztp[0:D, 0:128])
                        if layer < 2:
                            ph = psum_e.tile([128, 512], f32, tag="ph")
                            nc.tensor.matmul(
                                ph[:, 0:D], zts[:],
                                wt[:], start=True, stop=True)
                            h = epi.tile([128, D], f32, tag="h")
                            nc.vector.tensor_scalar_mul(
                                h[:], ph[:, 0:D], dinvt[:, t:t + 1])
                            nc.vector.tensor_add(
                                h[:], h[:],
                                b01t[:, layer * D:(layer + 1) * D])
                            hl = epi.tile([128, D], f32, tag="hl")
                            nc.scalar.mul(hl[:], h[:], NEG_SLOPE)
                            nc.vector.tensor_max(hl[:], hl[:], h[:])
                            nc.vector.tensor_scalar_mul(
                                ob[:, trel, :], hl[:], dinvt[:, t:t + 1])
                        else:
                            ph = psum_e.tile([128, 512], f32, tag="ph")
                            nc.tensor.matmul(
                                ph[:, 0:DOUT], zts[:],
                                wt[:], start=True, stop=True)
                            h = epi.tile([128, DOUT], f32, tag="h2")
                            nc.vector.tensor_scalar_mul(
                                h[:], ph[:, 0:DOUT], dinvt[:, t:t + 1])
                            nc.vector.tensor_add(ob[:, trel, :], h[:],
                                                 b2t[:])
                    dst = cc_in if layer < 2 else outr
                    nc.sync.dma_start(
                        dst[g * SUPN:(g + 1) * SUPN, :].rearrange(
                            "(a p) c -> p a c", p=128), ob[:])

                if layer < 2:
                    with tc.tile_critical():
                        cci = nc.gpsimd.collective_compute(
                            "AllGather", mybir.AluOpType.bypass,
                            ins=[cc_in[:, :].rearrange(
                                "(p two) c -> p (two c)", two=2)],
                            outs=[cc_out[layer][:, :]],
                            replica_groups=[list(range(cfg.W))])
                        cci.then_inc(cc_sem, 1)
                        cc_cnt[0] += 1
                        nc.gpsimd.wait_ge(cc_sem, cc_cnt[0])
                    if _DEBUG:
                        nc.sync.dma_start(dbg[layer][:],
                                          cc_out[layer][:])
    nc.compile()
    return nc


_CACHE = {}


def _get_program(key, cfg, edge_index):
    if key in _CACHE:
        return _CACHE[key]
    dinv, perms, sched, gidxs, sids = _preprocess(cfg, edge_index)
    nc = _build(cfg, sched)
    _CACHE[key] = (nc, dinv, perms, sched, gidxs, sids)
    return _CACHE[key]


def kernel(x, edge_index, W0, b0, W1, b1, W2, b2, _cfg=None, _sim=False):
    import ml_dtypes
    x = np.asarray(x, np.float32)
    edge_index = np.asarray(edge_index)
    N, D = x.shape
    DOUT = np.asarray(W2).shape[1]
    cfg = _cfg or _Cfg(N, D, DOUT)
    nc, dinv, perms, sched, gidxs, sids = _get_program(
        (N, edge_index.shape[1]), cfg, edge_index)

    BP, BLK, Wc, NTILES = cfg.BP, cfg.BLK, cfg.W, cfg.NTILES

    xs = (x * dinv[:, None]).astype(ml_dtypes.bfloat16)
    b01 = np.zeros((128, 2 * D), np.float32)
    b01[:, :D] = np.asarray(b0, np.float32)[None, :]
    b01[:, D:] = np.asarray(b1, np.float32)[None, :]
    b2t = np.tile(np.asarray(b2, np.float32)[None, :], (128, 1))
    iota = np.ascontiguousarray(
        np.tile(np.arange(128, dtype=np.float32)[None, :], (128, 1)))

    in_maps = []
    for c in range(Wc):
        lo, hi = c * BLK, min((c + 1) * BLK, N)
        db = np.zeros(BP, np.float32)
        db[perms[c]] = dinv[lo:hi]
        dinv_blk = np.ascontiguousarray(
            db.reshape(NTILES, 128).T).astype(np.float32)
        sid = sids[c].reshape(sched['nplanes'], 128).T
        im = dict(
            w0=np.asarray(W0, np.float32), w1=np.asarray(W1, np.float32),
            w2=np.asarray(W2, np.float32), b01=b01, b2b=b2t,
            dinv_blk=dinv_blk, iota_rep=iota,
            gidx=_wrap16(gidxs[c]),
            sid=np.ascontiguousarray(sid).astype(np.float32),
        )
        in_maps.append(im)

    # x_table is global (all cores see all blocks): assemble full table
    xt_full = np.zeros((cfg.NT, D), ml_dtypes.bfloat16)
    for c in range(Wc):
        lo, hi = c * BLK, min((c + 1) * BLK, N)
        xt_full[c * BP + perms[c]] = xs[lo:hi]
    xt_full = np.ascontiguousarray(xt_full.reshape(cfg.NPAIR, 128))
    for c in range(Wc):
        in_maps[c]["x_table"] = xt_full

    if _sim:
        from concourse import bass_interp
        sim = bass_interp.MultiCoreSim(nc, Wc)
        for c in range(Wc):
            for k, v in in_maps[c].items():
                sim.cores[c].tensor(k)[:] = v
            sim.cores[c].mem_tensor("outr")[:] = 0
        sim.simulate()
        results = [np.array(sim.cores[c].mem_tensor("outr")).reshape(BP, DOUT)
                   for c in range(Wc)]
    else:
        from concourse.bass_utils import run_bass_kernel_spmd
        res = run_bass_kernel_spmd(nc, in_maps, list(range(Wc)))
        results = [res.results[c]["outr"] for c in range(Wc)]

    out = np.zeros((N, DOUT), np.float32)
    for c in range(Wc):
        lo, hi = c * BLK, min((c + 1) * BLK, N)
        out[lo:hi] = results[c][perms[c]]
    return out
